# revision 6
# baseline (speedup 1.0000x reference)
"""MoE MLP (SequentialMLP over expert-grouped tokens) for 8x TRN2 NeuronCores.

Expert-parallel: core e owns expert e — its 2048-token group, w1[e], b1[e],
w2[e], b2[e].  Per core:   out = gelu_tanh(x @ w1 + b1) @ w2 + b2
with x [2048, 2048], w1 [2048, 8192], w2 [8192, 2048].

Device algorithm (per core, per 1024-token half):
  1. PE-transpose x half into xT [H=2048 on partitions, 1024 tokens]  (SBUF).
  2. For each supergroup of 4 f-tiles (512 ffn cols):
       GEMM1:  psum[f128, c512] = sum_h w1_tile^T @ xT    (float32r matmuls)
       gelu+b1 on ACT  ->  hT panel [f 512, c 1024] in SBUF
       GEMM2:  psum[c128, h'512] = sum_f hT_tile^T @ w2_tile  (+ b2 fold once)
       accumulate psum into SBUF out_acc via DVE copy/add.
  3. DMA out_acc -> out.
Weights stream from HBM exactly once per half (w1+w2 = 134 MB); xT and
out_acc stay SBUF-resident.  float32r streams fp32 data through the PE at
1 cycle/row (vs 4 for strict fp32), accumulating in fp32 PSUM.
"""
import sys

sys.path.insert(0, "/opt/trn_rl_repo")

from contextlib import ExitStack

import numpy as np

import concourse.bass as bass
import concourse.tile as tile
import concourse.mybir as mybir
from concourse import bacc
from concourse.bass_utils import run_bass_kernel_spmd
from concourse.masks import make_identity

F32 = mybir.dt.float32
F32R = mybir.dt.float32r
AFT = mybir.ActivationFunctionType

E, H, F, CAP = 8, 2048, 8192, 2048
HT = H // 128          # 16 h-tiles (GEMM1 contraction)
FT = F // 128          # 64 f-tiles
N_HALF = 2
CH = CAP // N_HALF     # 1024 tokens per half
CT_HALF = CH // 128    # 8 c-tiles per half
CC = 512               # GEMM1 moving-operand chunk (tokens)
NCC = CH // CC         # 2
SG_FT = 4              # f-tiles per supergroup
NSG = FT // SG_FT      # 16
HC = 512               # GEMM2 moving-operand chunk (h' cols)
NHC = H // HC          # 4


def build_nc(reps: int = 1):
    nc = bacc.Bacc("TRN2", target_bir_lowering=False, debug=False, num_devices=E)
    x_d = nc.dram_tensor("x", [CAP, H], F32, kind="ExternalInput").ap()
    w1_d = nc.dram_tensor("w1", [H, F], F32R, kind="ExternalInput").ap()
    b1_d = nc.dram_tensor("b1", [1, F], F32, kind="ExternalInput").ap()
    w2_d = nc.dram_tensor("w2", [F, H], F32R, kind="ExternalInput").ap()
    b2_d = nc.dram_tensor("b2", [1, H], F32R, kind="ExternalInput").ap()
    out_d = nc.dram_tensor("out", [CAP, H], F32, kind="ExternalOutput").ap()

    with tile.TileContext(nc) as tc, ExitStack() as ctx:
        const = ctx.enter_context(tc.tile_pool(name="const", bufs=1))
        xT_pool = ctx.enter_context(tc.tile_pool(name="xT", bufs=1))
        acc_pool = ctx.enter_context(tc.tile_pool(name="acc", bufs=1))
        hT_pool = ctx.enter_context(tc.tile_pool(name="hT", bufs=1))
        w1_pool = ctx.enter_context(tc.tile_pool(name="w1p", bufs=2))
        w2_pool = ctx.enter_context(tc.tile_pool(name="w2p", bufs=2))
        xs_pool = ctx.enter_context(tc.tile_pool(name="xs", bufs=2))
        pst = ctx.enter_context(tc.tile_pool(name="pst", bufs=2, space="PSUM"))
        ps1 = ctx.enter_context(tc.tile_pool(name="ps1", bufs=2, space="PSUM"))
        ps2 = ctx.enter_context(tc.tile_pool(name="ps2", bufs=3, space="PSUM"))

        ident = const.tile([128, 128], F32)
        make_identity(nc, ident[:])
        ones_st = const.tile([1, 128], F32)
        nc.gpsimd.memset(ones_st[:], 1.0)
        ones = const.tile([1, 128], F32R)
        nc.vector.tensor_copy(ones[:], ones_st[:])
        b2_sb = const.tile([1, H], F32R)
        nc.sync.dma_start(b2_sb[:], b2_d)
        # b1 arrives [1, F]; we need it per-partition ([f%128, ft]).  Load it
        # [ft, 128] (contiguous) and PE-transpose once.
        b1_stage = const.tile([FT, 128], F32)
        nc.sync.dma_start(b1_stage[:], b1_d.rearrange("a (t p) -> (a t) p", p=128))
        b1_sb = const.tile([128, FT], F32)
        pb = pst.tile([128, 512], F32, tag="pt")
        nc.tensor.transpose(pb[:, :FT], b1_stage[:], ident[:FT, :FT])
        nc.vector.tensor_copy(b1_sb[:], pb[:, :FT])

        for _rep in range(reps):
            for half in range(N_HALF):
                c0 = half * CH
                xT = xT_pool.tile([128, HT, CH], F32R)
                out_acc = acc_pool.tile([128, CT_HALF, H], F32)
                for ct in range(CT_HALF):
                    for xh in range(2):
                        xs = xs_pool.tile([128, H // 2], F32)
                        nc.sync.dma_start(
                            xs[:],
                            x_d[c0 + ct * 128 : c0 + (ct + 1) * 128,
                                xh * (H // 2) : (xh + 1) * (H // 2)],
                        )
                        for g in range(2):  # two psum groups of 4 transposes
                            pt = pst.tile([128, 512], F32, tag="pt")
                            for j in range(4):
                                nc.tensor.transpose(
                                    pt[:, j * 128 : (j + 1) * 128],
                                    xs[:, (g * 4 + j) * 128 : (g * 4 + j + 1) * 128],
                                    ident[:],
                                )
                            ht0 = xh * 8 + g * 4
                            nc.vector.tensor_copy(
                                xT[:, ht0 : ht0 + 4, ct * 128 : (ct + 1) * 128],
                                pt[:].rearrange("p (t c) -> p t c", t=4),
                            )

                for sg in range(NSG):
                    hT = hT_pool.tile([128, SG_FT, CH], F32R)
                    for fi in range(SG_FT):
                        ft = sg * SG_FT + fi
                        w1t = w1_pool.tile([128, HT, 128], F32R)
                        nc.sync.dma_start(
                            w1t[:],
                            w1_d[:, ft * 128 : (ft + 1) * 128].rearrange(
                                "(t p) f -> p t f", p=128
                            ),
                        )
                        for cc in range(NCC):
                            p1 = ps1.tile([128, CC], F32)
                            for ht in range(HT):
                                nc.tensor.matmul(
                                    p1[:],
                                    lhsT=w1t[:, ht, :],
                                    rhs=xT[:, ht, cc * CC : (cc + 1) * CC],
                                    start=(ht == 0),
                                    stop=(ht == HT - 1),
                                )
                            nc.scalar.activation(
                                hT[:, fi, cc * CC : (cc + 1) * CC],
                                p1[:],
                                AFT.Gelu_apprx_tanh,
                                bias=b1_sb[:, ft : ft + 1],
                            )

                    for hc in range(NHC):
                        w2t = w2_pool.tile([128, SG_FT, HC], F32R)
                        nc.sync.dma_start(
                            w2t[:],
                            w2_d[sg * SG_FT * 128 : (sg + 1) * SG_FT * 128,
                                 hc * HC : (hc + 1) * HC].rearrange(
                                "(t p) n -> p t n", p=128
                            ),
                        )
                        for ct in range(CT_HALF):
                            p2 = ps2.tile([128, HC], F32)
                            for fi in range(SG_FT):
                                nc.tensor.matmul(
                                    p2[:],
                                    lhsT=hT[:, fi, ct * 128 : (ct + 1) * 128],
                                    rhs=w2t[:, fi, :],
                                    start=(fi == 0),
                                    stop=(fi == SG_FT - 1 and sg > 0),
                                )
                            dst = out_acc[:, ct, hc * HC : (hc + 1) * HC]
                            if sg == 0:
                                nc.tensor.matmul(
                                    p2[:],
                                    lhsT=ones[:],
                                    rhs=b2_sb[:, hc * HC : (hc + 1) * HC],
                                    start=False,
                                    stop=True,
                                )
                                nc.vector.tensor_copy(dst, p2[:])
                            else:
                                nc.vector.tensor_add(dst, dst, p2[:])

                for ct in range(CT_HALF):
                    nc.sync.dma_start(
                        out_d[c0 + ct * 128 : c0 + (ct + 1) * 128, :],
                        out_acc[:, ct, :],
                    )
    nc.compile()
    return nc


def make_in_maps(inputs):
    hs = np.asarray(inputs["hidden_states"], dtype=np.float32)
    w1 = np.asarray(inputs["w1"], dtype=np.float32)
    b1 = np.asarray(inputs["b1"], dtype=np.float32)
    w2 = np.asarray(inputs["w2"], dtype=np.float32)
    b2 = np.asarray(inputs["b2"], dtype=np.float32)
    in_maps = []
    for e in range(E):
        in_maps.append({
            "x": np.ascontiguousarray(hs[e * CAP : (e + 1) * CAP]),
            "w1": np.ascontiguousarray(w1[e]),
            "b1": np.ascontiguousarray(b1[e]).reshape(1, F),
            "w2": np.ascontiguousarray(w2[e]),
            "b2": np.ascontiguousarray(b2[e]).reshape(1, H),
        })
    return in_maps


_NC_CACHE = {}


def _get_nc(reps=1):
    if reps not in _NC_CACHE:
        _NC_CACHE[reps] = build_nc(reps)
    return _NC_CACHE[reps]


def run_on_cores(inputs, reps=1):
    nc = _get_nc(reps)
    res = run_bass_kernel_spmd(nc, make_in_maps(inputs), core_ids=list(range(E)))
    return np.concatenate([r["out"] for r in res.results], axis=0)


def kernel(**inputs) -> np.ndarray:
    return run_on_cores(inputs, reps=1)


# revision 9
# speedup vs baseline: 1.4736x; 1.4736x over previous
"""MoE MLP (SequentialMLP over expert-grouped tokens) for 8x TRN2 NeuronCores.

Expert-parallel: core e owns expert e — its 2048-token group, w1[e], b1[e],
w2[e], b2[e].  Per core:   out = gelu_tanh(x @ w1 + b1) @ w2 + b2
with x [2048, 2048], w1 [2048, 8192], w2 [8192, 2048].

Device algorithm (per core, per 1024-token half):
  1. PE-transpose x half into xT [H=2048 on partitions, 1024 tokens]  (SBUF).
  2. For each supergroup of 4 f-tiles (512 ffn cols):
       GEMM1:  psum[f128, c512] = sum_h w1_tile^T @ xT    (float32r matmuls)
       gelu+b1 on ACT  ->  hT panel [f 512, c 1024] in SBUF
       GEMM2:  psum[c128, h'512] = sum_f hT_tile^T @ w2_tile  (+ b2 fold once)
       accumulate psum into SBUF out_acc via DVE copy/add.
  3. DMA out_acc -> out.
Weights stream from HBM exactly once per half (w1+w2 = 134 MB); xT and
out_acc stay SBUF-resident.  float32r streams fp32 data through the PE at
1 cycle/row (vs 4 for strict fp32), accumulating in fp32 PSUM.
"""
import sys

sys.path.insert(0, "/opt/trn_rl_repo")

from contextlib import ExitStack

import numpy as np

import concourse.bass as bass
import concourse.tile as tile
import concourse.mybir as mybir
from concourse import bacc
from concourse.bass_utils import run_bass_kernel_spmd
from concourse.masks import make_identity

F32 = mybir.dt.float32
F32R = mybir.dt.float32r
AFT = mybir.ActivationFunctionType

E, H, F, CAP = 8, 2048, 8192, 2048
HT = H // 128          # 16 h-tiles (GEMM1 contraction)
FT = F // 128          # 64 f-tiles
N_HALF = 2
CH = CAP // N_HALF     # 1024 tokens per half
CT_HALF = CH // 128    # 8 c-tiles per half
CC = 512               # GEMM1 moving-operand chunk (tokens)
NCC = CH // CC         # 2
SG_FT = 4              # f-tiles per supergroup
NSG = FT // SG_FT      # 16
HC = 512               # GEMM2 moving-operand chunk (h' cols)
NHC = H // HC          # 4


def build_nc(reps: int = 1):
    nc = bacc.Bacc("TRN2", target_bir_lowering=False, debug=False, num_devices=E)
    x_d = nc.dram_tensor("x", [CAP, H], F32, kind="ExternalInput").ap()
    w1_d = nc.dram_tensor("w1", [H, F], F32R, kind="ExternalInput").ap()
    b1_d = nc.dram_tensor("b1", [1, F], F32, kind="ExternalInput").ap()
    w2_d = nc.dram_tensor("w2", [F, H], F32R, kind="ExternalInput").ap()
    b2_d = nc.dram_tensor("b2", [1, H], F32R, kind="ExternalInput").ap()
    out_d = nc.dram_tensor("out", [CAP, H], F32, kind="ExternalOutput").ap()

    with tile.TileContext(nc) as tc, ExitStack() as ctx:
        const = ctx.enter_context(tc.tile_pool(name="const", bufs=1))
        xT_pool = ctx.enter_context(tc.tile_pool(name="xT", bufs=1))
        acc_pool = ctx.enter_context(tc.tile_pool(name="acc", bufs=1))
        hT_pool = ctx.enter_context(tc.tile_pool(name="hT", bufs=1))
        w1_pool = ctx.enter_context(tc.tile_pool(name="w1p", bufs=2))
        w2_pool = ctx.enter_context(tc.tile_pool(name="w2p", bufs=2))
        xs_pool = ctx.enter_context(tc.tile_pool(name="xs", bufs=2))
        b2_pool = ctx.enter_context(tc.tile_pool(name="b2p", bufs=2))
        pst = ctx.enter_context(tc.tile_pool(name="pst", bufs=2, space="PSUM"))
        ps1 = ctx.enter_context(tc.tile_pool(name="ps1", bufs=2, space="PSUM"))
        ps2 = ctx.enter_context(tc.tile_pool(name="ps2", bufs=4, space="PSUM"))

        ident = const.tile([128, 128], F32)
        make_identity(nc, ident[:])
        ones_st = const.tile([1, 128], F32)
        nc.gpsimd.memset(ones_st[:], 1.0)
        ones = const.tile([1, 128], F32R)
        nc.vector.tensor_copy(ones[:], ones_st[:])
        # b1 arrives [1, F]; we need it per-partition ([f%128, ft]).  Load it
        # [ft, 128] (contiguous) and PE-transpose once.
        b1_stage = const.tile([FT, 128], F32)
        nc.sync.dma_start(b1_stage[:], b1_d.rearrange("a (t p) -> (a t) p", p=128))
        b1_sb = const.tile([128, FT], F32)
        pb = pst.tile([128, 512], F32, tag="pt")
        nc.tensor.transpose(pb[:, :FT], b1_stage[:], ident[:FT, :FT])
        nc.vector.tensor_copy(b1_sb[:], pb[:, :FT])

        for _rep in range(reps):
            for half in range(N_HALF):
                c0 = half * CH
                xT = xT_pool.tile([128, HT, CH], F32R)
                out_acc = acc_pool.tile([128, CT_HALF, H], F32)
                for ct in range(CT_HALF):
                    for xh in range(2):
                        xs = xs_pool.tile([128, H // 2], F32)
                        nc.sync.dma_start(
                            xs[:],
                            x_d[c0 + ct * 128 : c0 + (ct + 1) * 128,
                                xh * (H // 2) : (xh + 1) * (H // 2)],
                        )
                        for g in range(2):  # two psum groups of 4 transposes
                            pt = pst.tile([128, 512], F32, tag="pt")
                            for j in range(4):
                                nc.tensor.transpose(
                                    pt[:, j * 128 : (j + 1) * 128],
                                    xs[:, (g * 4 + j) * 128 : (g * 4 + j + 1) * 128],
                                    ident[:],
                                )
                            ht0 = xh * 8 + g * 4
                            nc.vector.tensor_copy(
                                xT[:, ht0 : ht0 + 4, ct * 128 : (ct + 1) * 128],
                                pt[:].rearrange("p (t c) -> p t c", t=4),
                            )

                for sg in range(NSG):
                    hT = hT_pool.tile([128, SG_FT, CH], F32R)
                    FW = SG_FT * 128
                    w1ab = []
                    for ha in range(2):
                        w1t = w1_pool.tile([128, HT // 2, FW], F32R, tag="w1t",
                                           name=f"w1t_{half}_{sg}_{ha}")
                        nc.sync.dma_start(
                            w1t[:],
                            w1_d[ha * (H // 2) : (ha + 1) * (H // 2),
                                 sg * FW : (sg + 1) * FW].rearrange(
                                "(t p) f -> p t f", p=128
                            ),
                        )
                        w1ab.append(w1t)
                    for fi in range(SG_FT):
                        ft = sg * SG_FT + fi
                        for cc in range(NCC):
                            p1 = ps1.tile([128, CC], F32)
                            for ht in range(HT):
                                nc.tensor.matmul(
                                    p1[:],
                                    lhsT=w1ab[ht // 8][:, ht % 8,
                                                       fi * 128 : (fi + 1) * 128],
                                    rhs=xT[:, ht, cc * CC : (cc + 1) * CC],
                                    start=(ht == 0),
                                    stop=(ht == HT - 1),
                                )
                            nc.scalar.activation(
                                hT[:, fi, cc * CC : (cc + 1) * CC],
                                p1[:],
                                AFT.Gelu_apprx_tanh,
                                bias=b1_sb[:, ft : ft + 1],
                            )

                    for hc in range(NHC):
                        w2t = w2_pool.tile([128, SG_FT, HC], F32R)
                        nc.sync.dma_start(
                            w2t[:],
                            w2_d[sg * SG_FT * 128 : (sg + 1) * SG_FT * 128,
                                 hc * HC : (hc + 1) * HC].rearrange(
                                "(t p) n -> p t n", p=128
                            ),
                        )
                        for ct in range(CT_HALF):
                            p2 = ps2.tile([128, HC], F32)
                            for fi in range(SG_FT):
                                nc.tensor.matmul(
                                    p2[:],
                                    lhsT=hT[:, fi, ct * 128 : (ct + 1) * 128],
                                    rhs=w2t[:, fi, :],
                                    start=(fi == 0),
                                    stop=(fi == SG_FT - 1 and sg > 0),
                                )
                            dst = out_acc[:, ct, hc * HC : (hc + 1) * HC]
                            if sg == 0:
                                if ct == 0:
                                    b2_sb = b2_pool.tile([1, HC], F32R, tag="b2",
                                                         name=f"b2_{half}_{hc}")
                                    nc.sync.dma_start(
                                        b2_sb[:], b2_d[:, hc * HC : (hc + 1) * HC])
                                nc.tensor.matmul(
                                    p2[:],
                                    lhsT=ones[:],
                                    rhs=b2_sb[:],
                                    start=False,
                                    stop=True,
                                )
                                nc.vector.tensor_copy(dst, p2[:])
                            else:
                                nc.vector.tensor_add(dst, dst, p2[:])

                for ct in range(CT_HALF):
                    nc.sync.dma_start(
                        out_d[c0 + ct * 128 : c0 + (ct + 1) * 128, :],
                        out_acc[:, ct, :],
                    )
    nc.compile()
    return nc


def make_in_maps(inputs):
    hs = np.asarray(inputs["hidden_states"], dtype=np.float32)
    w1 = np.asarray(inputs["w1"], dtype=np.float32)
    b1 = np.asarray(inputs["b1"], dtype=np.float32)
    w2 = np.asarray(inputs["w2"], dtype=np.float32)
    b2 = np.asarray(inputs["b2"], dtype=np.float32)
    in_maps = []
    for e in range(E):
        in_maps.append({
            "x": np.ascontiguousarray(hs[e * CAP : (e + 1) * CAP]),
            "w1": np.ascontiguousarray(w1[e]),
            "b1": np.ascontiguousarray(b1[e]).reshape(1, F),
            "w2": np.ascontiguousarray(w2[e]),
            "b2": np.ascontiguousarray(b2[e]).reshape(1, H),
        })
    return in_maps


_NC_CACHE = {}


def _get_nc(reps=1):
    if reps not in _NC_CACHE:
        _NC_CACHE[reps] = build_nc(reps)
    return _NC_CACHE[reps]


def run_on_cores(inputs, reps=1):
    nc = _get_nc(reps)
    res = run_bass_kernel_spmd(nc, make_in_maps(inputs), core_ids=list(range(E)))
    return np.concatenate([r["out"] for r in res.results], axis=0)


def kernel(**inputs) -> np.ndarray:
    return run_on_cores(inputs, reps=1)
